# revision 4
# baseline (speedup 1.0000x reference)
"""AnchorAttention Trainium2 kernel, SPMD over 8 NeuronCores — head-split.

Sharding: core i -> (batch b = i//2, head half j = i%2).  Each core
processes ALL 4096 tokens of its batch for its 8 heads: QKV/Q
column-parallel, proj row-parallel; the host sums the two proj partials
per batch and adds bproj.

v2 changes vs v1 (263µs baseline):
  - Scores for each (a-tile, head-pair) land in ONE [128,1024] f32 PSUM
    tile ([pse|pso], 2 banks); a single Exp ACTIVATE covers both heads,
    halving ScalarE instruction count (the v1 steady-state pacer).
  - AV even/odd accumulate into one [128,1024] PSUM tile; one
    reciprocal over [64,1024] covers both denominators.
  - PSUM laid out by tag: ps2 2x2 banks, av 1x2, mm 2x1 = 8 banks, with
    the per-step PE issue order arranged so score-bank reuse never
    waits on the Exp that frees it.
  - All DMA moved to the sync-engine HWDGE queue (gpsimd issues
    nothing), batched: one dma_start per x block [128,8x512], weights
    in halves, y stores as [128,1024] pairs.  This removes v1's 10.8µs
    software-DGE drain + late y-issue serialization in the tail.
"""

import sys
from contextlib import ExitStack

sys.path.insert(0, "/opt/trn_rl_repo")

import ml_dtypes
import numpy as np

import concourse.bass as bass
import concourse.mybir as mybir
import concourse.tile as tile
from concourse import bacc
from concourse.bass_utils import run_bass_kernel_spmd

F32 = mybir.dt.float32
BF16 = mybir.dt.bfloat16

B, S, DIM = 4, 4096, 1024
H, D = 16, 64
A = 512              # anchor tokens
TOK = 4096           # tokens per core (full batch)
NBLK = 8             # 512-token blocks per core
BLK = 512
N_CORES = 8
SCALE = 1.0 / np.sqrt(D)

HD = DIM // 2        # qk/v dims per core (8 heads x 64)
KQ = HD // 128       # 4 qk-dim tiles per core
KD = DIM // 128      # 8 contraction tiles (x width)
NA = A // 128        # 4 anchor tiles
NP = H // 4          # 4 head pairs per core

_COMPILED = {}


def build_kernel():
    nc = bacc.Bacc(trn_type="TRN2", target_bir_lowering=False)

    xT = nc.declare_dram_parameter("xT", [DIM, TOK], BF16, isOutput=False)
    wk = nc.declare_dram_parameter("wk", [DIM, HD], BF16, isOutput=False)
    wv = nc.declare_dram_parameter("wv", [DIM, HD], BF16, isOutput=False)
    wqa = nc.declare_dram_parameter("wqa", [DIM, HD], BF16, isOutput=False)
    wqb = nc.declare_dram_parameter("wqb", [DIM, HD], BF16, isOutput=False)
    wproj = nc.declare_dram_parameter("wproj", [HD, DIM], BF16, isOutput=False)
    y = nc.declare_dram_parameter("y", [TOK, DIM], BF16, isOutput=True)

    with tile.TileContext(nc) as tc, ExitStack() as ctx:
        const = ctx.enter_context(tc.tile_pool(name="const", bufs=1))
        p_w = ctx.enter_context(tc.tile_pool(name="p_w", bufs=1))
        p_kt = ctx.enter_context(tc.tile_pool(name="p_kt", bufs=1))
        p_v = ctx.enter_context(tc.tile_pool(name="p_v", bufs=1))
        p_xt = ctx.enter_context(tc.tile_pool(name="p_xt", bufs=3))
        p_qt = ctx.enter_context(tc.tile_pool(name="p_qt", bufs=8))
        p_exp = ctx.enter_context(tc.tile_pool(name="p_exp", bufs=8))
        p_rb = ctx.enter_context(tc.tile_pool(name="p_rb", bufs=2))
        p_ot = ctx.enter_context(tc.tile_pool(name="p_ot", bufs=8))
        p_y = ctx.enter_context(tc.tile_pool(name="p_y", bufs=4))
        # PSUM: ps2 2x[128,1024] (4 banks) + av 1x[128,1024] (2 banks)
        #       + mm 2x[128,512] (2 banks) = 8 banks exactly.
        p_ps = ctx.enter_context(tc.tile_pool(name="p_ps", bufs=1, space="PSUM"))

        def ps2_tile(name):
            return p_ps.tile([128, 1024], F32, tag="ps2", bufs=2, name=name)

        def av_tile(name):
            return p_ps.tile([128, 1024], F32, tag="av", bufs=1, name=name)

        def mm_tile(name):
            return p_ps.tile([128, BLK], F32, tag="mm", bufs=2, name=name)

        # ---- PE warm-up while the first DMAs land ----
        warm = const.tile([128, 512], BF16, tag="warm")
        nc.vector.memset(warm[:], 0.0)
        wps = av_tile("warmps")
        NWARM = 20
        for i in range(NWARM):
            nc.tensor.matmul(
                wps[:, 0:512], warm[:, 0:128], warm[:],
                start=(i == 0), stop=(i == NWARM - 1),
            )

        # ---- input DMAs: batched, alternating sync/scalar HWDGE queues.
        # The anchor tokens ARE the first x block, so xt0 doubles as aT. ----
        xt_sb = [None] * NBLK

        def issue_xt(blk, eng, split=False):
            t = p_xt.tile([128, KD * BLK], BF16, tag="xt", name=f"xt{blk}")
            src = xT[:, BLK * blk : BLK * (blk + 1)].rearrange(
                "(k p) t -> p k t", p=128
            )
            dst = t[:].rearrange("p (k t) -> p k t", t=BLK)
            if split:
                nc.sync.dma_start(dst[:, 0 : KD // 2], src[:, 0 : KD // 2])
                nc.scalar.dma_start(dst[:, KD // 2 :], src[:, KD // 2 :])
            else:
                eng.dma_start(dst, src)
            xt_sb[blk] = t

        def load_w(dram, name, eng, halves=False):
            dim0, dim1 = dram.shape
            k = dim0 // 128
            t = p_w.tile([128, k * dim1], BF16, tag=name, name=name)
            src = dram[:, :].rearrange("(k p) c -> p k c", p=128)
            dst = t[:].rearrange("p (k c) -> p k c", c=dim1)
            if halves:
                eng.dma_start(dst[:, 0 : k // 2], src[:, 0 : k // 2])
                eng.dma_start(dst[:, k // 2 :], src[:, k // 2 :])
            else:
                eng.dma_start(dst, src)
            return t

        issue_xt(0, None, split=True)
        wk_sb = load_w(wk, "wk", nc.scalar, halves=True)     # [128, 8*512]
        wv_sb = load_w(wv, "wv", nc.sync)
        wqa_sb = load_w(wqa, "wqa", nc.scalar)
        issue_xt(1, nc.sync)
        wqb_sb = load_w(wqb, "wqb", nc.scalar)
        wp_sb = load_w(wproj, "wp", nc.scalar)               # [128, 4*1024]

        def xt_c(blk, k):  # [128,512] chunk k of block blk
            return xt_sb[blk][:, BLK * k : BLK * (k + 1)]

        def w_c(t, k):     # [128,512] chunk k of a packed qkv weight
            return t[:, BLK * k : BLK * (k + 1)]

        # ---- KT[qk, a] = Wk^T aT, m-major so the first CAST fires early ----
        ktps = [ps2_tile(f"ktps{h}") for h in range(2)]
        for m in range(KQ):
            for k in range(KD):
                nc.tensor.matmul(
                    ktps[m // 2][:, 512 * (m % 2) : 512 * (m % 2) + 512],
                    w_c(wk_sb, k)[:, 128 * m : 128 * (m + 1)],
                    xt_c(0, k),
                    start=(k == 0), stop=(k == KD - 1),
                )
        # packed kt: kt_sb[h] = [kt(2h) | kt(2h+1)], each [128, A]
        kt_sb = []
        for h in range(2):
            kt = p_kt.tile([128, 2 * A], BF16, name=f"kt{h}", tag=f"kt{h}")
            nc.vector.tensor_copy(kt[:], ktps[h][:])
            kt_sb.append(kt)

        def kt_c(i):       # [128, A] view for head pair i
            return kt_sb[i // 2][:, A * (i % 2) : A * (i % 2) + A]

        # ---- V (a-major), then packed [ones|V_even|ones|V_odd] tiles ----
        vps = [ps2_tile(f"vps{h}") for h in range(2)]
        for a in range(NA):
            for k in range(KD):
                nc.tensor.matmul(
                    vps[a // 2][:, 512 * (a % 2) : 512 * (a % 2) + 512],
                    xt_c(0, k)[:, 128 * a : 128 * (a + 1)],
                    w_c(wv_sb, k),
                    start=(k == 0), stop=(k == KD - 1),
                )
        v_sb = []
        for a in range(NA):
            t = p_v.tile([128, 2 * HD], BF16, name=f"v{a}", tag=f"v{a}")
            nc.vector.memset(
                t[:].rearrange("p (hp c) -> p hp c", c=2 * D)[:, :, 0:D], 1.0
            )
            v_sb.append(t)
        for a in range(NA):
            vr = v_sb[a][:].rearrange("p (hp c) -> p hp c", c=4 * D)
            pr = vps[a // 2][:, 512 * (a % 2) : 512 * (a % 2) + 512].rearrange(
                "p (hp c) -> p hp c", c=2 * D
            )
            nc.vector.tensor_copy(vr[:, :, D : 2 * D], pr[:, :, 0:D])
            nc.vector.tensor_copy(vr[:, :, 3 * D : 4 * D], pr[:, :, D : 2 * D])

        # ---- Q-proj of block 0 (anchors: wqa), m-major ----
        qt_sb = [[None] * KQ for _ in range(NBLK)]
        qps = [ps2_tile(f"q0ps{h}") for h in range(2)]
        for m in range(KQ):
            for k in range(KD):
                nc.tensor.matmul(
                    qps[m // 2][:, 512 * (m % 2) : 512 * (m % 2) + 512],
                    w_c(wqa_sb, k)[:, 128 * m : 128 * (m + 1)],
                    xt_c(0, k),
                    start=(k == 0), stop=(k == KD - 1),
                )
        for m in range(KQ):
            qt = p_qt.tile([128, BLK], BF16, tag="qt")
            nc.vector.tensor_copy(qt[:], qps[m // 2][:, 512 * (m % 2) : 512 * (m % 2) + 512])
            qt_sb[0][m] = qt

        ot_sb = [[None] * KQ for _ in range(NBLK)]

        def emit_score_tile(blk, i, a):
            """One [128,1024] PSUM tile = [pse(a)|pso(a)] for head pair i;
            one Exp ACTIVATE over both halves -> e tile [128,1024] bf16."""
            ps = ps2_tile(f"sc{blk}_{i}_{a}")
            nc.tensor.matmul(
                ps[:, 0:512],
                kt_c(i)[0:D, 128 * a : 128 * (a + 1)],
                qt_sb[blk][i][0:D, :],
                start=True, stop=True,
                tile_position=(0, 0),
            )
            nc.tensor.matmul(
                ps[:, 512:1024],
                kt_c(i)[D : 2 * D, 128 * a : 128 * (a + 1)],
                qt_sb[blk][i][D : 2 * D, :],
                start=True, stop=True,
                tile_position=(64, 0),
            )
            e = p_exp.tile([128, 1024], BF16, tag="exp")
            nc.scalar.activation(
                e[:], ps[:], mybir.ActivationFunctionType.Exp, scale=float(SCALE)
            )
            return e

        def emit_av_pair(blk, q, e_tiles):
            av = av_tile(f"av{blk}_{q}")
            for a in range(NA):
                nc.tensor.matmul(
                    av[:, 0:512],
                    v_sb[a][:, 256 * q : 256 * q + 128],
                    e_tiles[a][:, 0:512],
                    start=(a == 0), stop=(a == NA - 1),
                )
            for a in range(NA):
                nc.tensor.matmul(
                    av[:, 512:1024],
                    v_sb[a][:, 256 * q + 128 : 256 * (q + 1)],
                    e_tiles[a][:, 512:1024],
                    start=(a == 0), stop=(a == NA - 1),
                )
            rb = p_rb.tile([128, 1024], F32, tag="rb")
            nc.vector.reciprocal_approx_fast(rb[0:D, :], av[0:D, :])
            nc.vector.tensor_mul(
                ot_sb[blk][q][0:D, :], av[D : 2 * D, 0:512], rb[0:D, 0:512]
            )
            nc.vector.tensor_mul(
                ot_sb[blk][q][D : 2 * D, :], av[D : 2 * D, 512:1024], rb[0:D, 512:1024]
            )

        def emit_q_mtile(blk, m):
            ps = mm_tile(f"qp{blk}_{m}")
            for k in range(KD):
                nc.tensor.matmul(
                    ps[:], w_c(wqb_sb, k)[:, 128 * m : 128 * (m + 1)], xt_c(blk, k),
                    start=(k == 0), stop=(k == KD - 1),
                )
            qt = p_qt.tile([128, BLK], BF16, tag="qt")
            nc.vector.tensor_copy(qt[:], ps[:])
            qt_sb[blk][m] = qt

        yt_cur = [None]

        def emit_proj_tile(blk, idx, evac=None):
            """idx = 2*tt + n.  n=0 allocates yt [128,1024]; n=1 DMAs it."""
            tt, n = idx // 2, idx % 2
            ps = mm_tile(f"pj{blk}_{idx}")
            for k2 in range(KQ):
                nc.tensor.matmul(
                    ps[:],
                    ot_sb[blk][k2][:, 128 * tt : 128 * (tt + 1)],
                    wp_sb[:, 1024 * k2 + 512 * n : 1024 * k2 + 512 * (n + 1)],
                    start=(k2 == 0), stop=(k2 == KQ - 1),
                )
            if n == 0:
                yt_cur[0] = p_y.tile(
                    [128, 1024], BF16, tag="y", name=f"yt{blk}_{tt}"
                )
            yt = yt_cur[0]
            if evac is None:
                nc.vector.tensor_copy(yt[:, 512 * n : 512 * (n + 1)], ps[:])
            else:
                nc.scalar.copy(yt[:, 512 * n : 512 * (n + 1)], ps[:])
            if n == 1:
                nc.sync.dma_start(
                    y[BLK * blk + 128 * tt : BLK * blk + 128 * (tt + 1), :],
                    yt[:],
                )

        # ---- steady state: 8 blocks x 4 software-pipelined pair-steps.
        # Per-step PE order keeps >=1.6µs between reuses of each ps2 slot. ----
        for blk in range(NBLK):
            if blk + 2 < NBLK:
                issue_xt(blk + 2, nc.sync)
            for q in range(KQ):
                ot_sb[blk][q] = p_ot.tile(
                    [128, BLK], BF16, tag="ot", name=f"ot{blk}_{q}"
                )
            prev = None
            for i in range(NP):
                has_qp = blk + 1 < NBLK
                has_pj = blk > 0
                e_tiles = [None] * NA
                e_tiles[0] = emit_score_tile(blk, i, 0)
                e_tiles[1] = emit_score_tile(blk, i, 1)
                if prev is not None:
                    emit_av_pair(blk, prev[0], prev[1])
                    e_tiles[2] = emit_score_tile(blk, i, 2)
                    if has_qp:
                        emit_q_mtile(blk + 1, i)
                    elif has_pj:
                        emit_proj_tile(blk - 1, 2 * i)
                    e_tiles[3] = emit_score_tile(blk, i, 3)
                    if has_pj:
                        if has_qp:
                            emit_proj_tile(blk - 1, 2 * i)
                        emit_proj_tile(blk - 1, 2 * i + 1)
                else:
                    # first step of a block: no AV to space out the ps2
                    # slot reuse — use qproj/proj between score tiles
                    if has_qp:
                        emit_q_mtile(blk + 1, i)
                    elif has_pj:
                        emit_proj_tile(blk - 1, 2 * i)
                    e_tiles[2] = emit_score_tile(blk, i, 2)
                    if has_pj and not has_qp:
                        emit_proj_tile(blk - 1, 2 * i + 1)
                    e_tiles[3] = emit_score_tile(blk, i, 3)
                    if has_pj and has_qp:
                        emit_proj_tile(blk - 1, 2 * i)
                        emit_proj_tile(blk - 1, 2 * i + 1)
                prev = (i, e_tiles)
            emit_av_pair(blk, prev[0], prev[1])

        # ---- tail: out-proj of the last block ----
        for idx in range(8):
            emit_proj_tile(NBLK - 1, idx, evac=("scalar" if idx % 2 else None))

    nc.compile()
    return nc


def _shard_inputs(x, Wqkv, Wq, Wproj):
    """Per-core inputs: core i -> (batch i//2, head half i%2)."""
    x = np.asarray(x, dtype=np.float32)
    Wqkv = np.asarray(Wqkv, dtype=np.float32)
    Wq = np.asarray(Wq, dtype=np.float32)
    Wproj = np.asarray(Wproj, dtype=np.float32)

    bf16 = ml_dtypes.bfloat16
    halves = []
    for j in range(2):
        hs = slice(HD * j, HD * (j + 1))
        halves.append(
            {
                "wk": np.ascontiguousarray(Wqkv[:, DIM : 2 * DIM][:, hs]).astype(bf16),
                "wv": np.ascontiguousarray(Wqkv[:, 2 * DIM :][:, hs]).astype(bf16),
                "wqa": np.ascontiguousarray(Wqkv[:, :DIM][:, hs]).astype(bf16),
                "wqb": np.ascontiguousarray(Wq[:, hs]).astype(bf16),
                "wproj": np.ascontiguousarray(Wproj[hs, :]).astype(bf16),
            }
        )
    in_maps = []
    for core in range(N_CORES):
        b, j = core // 2, core % 2
        m = dict(halves[j])
        m["xT"] = np.ascontiguousarray(x[b].T).astype(bf16)
        in_maps.append(m)
    return in_maps


def kernel(x, Wqkv, bqkv, Wq, bq, Wproj, bproj, num_anchor_tokens, **run_kwargs):
    assert int(num_anchor_tokens) == A
    if "nc" not in _COMPILED:
        _COMPILED["nc"] = build_kernel()
    nc = _COMPILED["nc"]
    in_maps = _shard_inputs(x, Wqkv, Wq, Wproj)
    res = run_bass_kernel_spmd(
        nc, in_maps, core_ids=list(range(N_CORES)), **run_kwargs
    )
    bproj = np.asarray(bproj, dtype=np.float32)
    out = np.empty((B, S, DIM), dtype=np.float32)
    for b in range(B):
        out[b] = np.asarray(res.results[2 * b]["y"], dtype=np.float32)
        out[b] += np.asarray(res.results[2 * b + 1]["y"], dtype=np.float32)
    out += bproj[None, None, :]
    _COMPILED["last_result"] = res
    return out
